# revision 1
# baseline (speedup 1.0000x reference)
"""Grouped Query Attention on 8 TRN2 NeuronCores.

Sharding: batch x s_q-quarter (core c -> batch c//4, query rows
[512*(c%4), 512*(c%4+1))). Each core computes the Q projection for its
512 query rows, attention for all 16 heads over its query rows, and the
output projection for a disjoint [512, 2048] slice of the output.

The KV projection is sharded: each core projects K^T and V only for its
OWN sequence quarter (= chunk 0 of its rotated x), packs them into a
1 MB DRAM buffer, and a 4-core AllGather per batch assembles the full
K^T/V in canonical sequence order while the tensor engine runs the Q
projection. Attention consumes the gathered K/V (s_k order is
permutation-invariant; K and V share the canonical order).

Other structure (v2):
- All matmul inputs bf16; PSUM accumulation f32.
- x chunk / Q^T / K^T / V / attn outputs are SBUF-resident.
- V is projected directly in [s, d] orientation (lhsT = x^T s-tile) so
  phase 2 needs no PE transposes.
- Scores land in [P, 2, 512] PSUM tiles so each ACT exp instruction
  covers 1024 columns.
- Per-head A-pass (scores+exp) / B-pass (attnV+denominator) software
  pipeline keeps the tensor engine dense (full p-state clock).
- Normalization: ones-matmul denominator -> DVE reciprocal -> GPSIMD
  partition_broadcast -> DVE multiply.
- Bulk weight loads ride the ACT-engine DMA queue so the SP queue only
  carries the latency-critical stream (x chunk, per-head Q weights).
- 1/sqrt(128) folded into Wq on host.
"""

import numpy as np

E = 2048
S = 2048
P = 128
H = 16
G = 4
SQ = 512          # query rows per core
EB = E // P       # 16 e-blocks (contraction tiles)
KV_N = 2 * E // G  # 1024
NCORES = 8

_NC = None
TRACE = False
LAST_RESULT = None


def _build():
    import concourse.bacc as bacc
    import concourse.mybir as mybir
    import concourse.tile as tile

    f32 = mybir.dt.float32
    bf16 = mybir.dt.bfloat16
    EXP = mybir.ActivationFunctionType.Exp
    IDENT = mybir.ActivationFunctionType.Identity

    nc = bacc.Bacc("TRN2", target_bir_lowering=False, debug=False,
                   num_devices=NCORES)

    # host layouts:
    #   xt:  x^T rotated chunk 0 (this core's quarter), [hd, eb, s_own]
    #   wq:  [head, p, eb, p] (1/sqrt(d) folded)
    #   wkv: [p, eb, 1024] with columns [K0 K1 K2 K3 V0 V1 V2 V3]
    #   wo:  [p, eb, e]
    xt = nc.declare_dram_parameter("xt", [P, EB, SQ], bf16, isOutput=False).ap()
    wq = nc.declare_dram_parameter("wq", [H, P, EB, P], bf16, isOutput=False).ap()
    wkv = nc.declare_dram_parameter("wkv", [P, EB, KV_N], bf16, isOutput=False).ap()
    wo = nc.declare_dram_parameter("wo", [P, EB, E], bf16, isOutput=False).ap()
    bq = nc.declare_dram_parameter("bq", [P, H], f32, isOutput=False).ap()
    bkvk = nc.declare_dram_parameter("bkvk", [P, 4], f32, isOutput=False).ap()
    bkvv = nc.declare_dram_parameter("bkvv", [1, 512], bf16, isOutput=False).ap()
    bo = nc.declare_dram_parameter("bo", [1, E], bf16, isOutput=False).ap()
    out = nc.declare_dram_parameter("out", [SQ, E], f32, isOutput=True).ap()

    RG = [[0, 1, 2, 3], [4, 5, 6, 7]]

    with tile.TileContext(nc) as tc:
        with tc.tile_pool(name="consts", bufs=1) as cp, \
             tc.tile_pool(name="qtsp", bufs=1) as qtsp, \
             tc.tile_pool(name="kvp", bufs=1) as kvp, \
             tc.tile_pool(name="otp", bufs=1) as otp, \
             tc.tile_pool(name="dram", bufs=1, space="DRAM") as dp:
            onec = cp.tile([P, 1], bf16, tag="onec")
            nc.vector.memset(onec, 1.0)
            oner = cp.tile([1, P], bf16, tag="oner")
            nc.vector.memset(oner, 1.0)
            # consts ride the ACT queue so the SP queue starts on x at once
            bq_s = cp.tile([P, H], f32, tag="bqs")
            nc.scalar.dma_start(bq_s, bq)
            bkvk_s = cp.tile([P, 4], f32, tag="bkvks")
            nc.scalar.dma_start(bkvk_s, bkvk)
            bkvv_s = cp.tile([1, 512], bf16, tag="bkvvs")
            nc.scalar.dma_start(bkvv_s, bkvv)
            bo_s = cp.tile([1, E], bf16, tag="bos")
            nc.scalar.dma_start(bo_s, bo)

            qts = qtsp.tile([P, H, SQ], bf16, tag="qts")    # Q^T, [hd, head, sq]
            kts = kvp.tile([P, G, S], bf16, tag="kts")      # K^T, [hd, group, sk]
            vgs = kvp.tile([P, EB, 512], bf16, tag="vgs")   # V, [sk, sk_tile, g*128+hd]
            OT = otp.tile([P, H, SQ], bf16, tag="ot")       # attn out, [hd, head, sq]

            # own-quarter KV pack: m 0..3 = K^T groups, m 4..7 = V s-tiles
            kvown = dp.tile([P, 8, 512], bf16, tag="kvown")
            kvall = dp.tile([4, P, 8, 512], bf16, tag="kvall")

            # ---- Phase 1: projections from the SBUF-resident x^T quarter.
            with tc.tile_pool(name="xsp", bufs=1) as xsp, \
                 tc.tile_pool(name="wkvp", bufs=1) as wkvp, \
                 tc.tile_pool(name="kvsg", bufs=1) as kvsg, \
                 tc.tile_pool(name="wqp", bufs=2) as wqp, \
                 tc.tile_pool(name="ps1", bufs=3, space="PSUM") as ps1, \
                 tc.tile_pool(name="ps1b", bufs=3, space="PSUM") as ps1b:
                xs = xsp.tile([P, EB, SQ], bf16, tag="xs")
                # split so the first e-blocks land quickly
                for c4 in range(4):
                    nc.sync.dma_start(xs[:, 4 * c4:4 * (c4 + 1)],
                                      xt[:, 4 * c4:4 * (c4 + 1)])
                wkv_s = wkvp.tile([P, EB, KV_N], bf16, tag="wkvs")
                nc.scalar.dma_start(wkv_s[:, :, 0:512], wkv[:, :, 0:512])
                nc.scalar.dma_start(wkv_s[:, :, 512:KV_N], wkv[:, :, 512:KV_N])
                kvstg = kvsg.tile([P, 8, 512], bf16, tag="kvstg")

                def q_head(m):
                    wqm = wqp.tile([P, EB, P], bf16, tag="wqm")
                    nc.sync.dma_start(wqm, wq[m])
                    ps = ps1.tile([P, SQ], f32, tag="ps")
                    for b in range(EB):
                        nc.tensor.matmul(ps, wqm[:, b], xs[:, b],
                                         start=(b == 0), stop=(b == EB - 1))
                    nc.vector.tensor_scalar_add(qts[:, m], ps, bq_s[:, m:m + 1])

                # two Q heads first so the PE starts immediately
                q_head(0)
                q_head(1)

                # K^T for all 4 groups over this core's own quarter
                for m in range(G):
                    ps = ps1b.tile([P, 512], f32, tag="ps")
                    for b in range(EB):
                        nc.tensor.matmul(
                            ps, wkv_s[:, b, m * P:(m + 1) * P], xs[:, b],
                            start=(b == 0), stop=(b == EB - 1))
                    nc.scalar.activation(kvstg[:, m], ps, IDENT,
                                         bias=bkvk_s[:, m:m + 1])

                # V in [s, d] orientation for this core's own 4 s-tiles
                for t in range(4):
                    ps = ps1b.tile([P, 512], f32, tag="ps")
                    nc.tensor.matmul(ps, oner, bkvv_s, start=True, stop=False)
                    for b in range(EB):
                        nc.tensor.matmul(
                            ps, xs[:, b, t * P:(t + 1) * P],
                            wkv_s[:, b, 512:KV_N],
                            start=False, stop=(b == EB - 1))
                    nc.vector.tensor_copy(kvstg[:, 4 + t], ps)

                # pack -> DRAM -> AllGather (runs while Q projection continues)
                nc.sync.dma_start(kvown, kvstg)
                nc.gpsimd.collective_compute(
                    "AllGather", mybir.AluOpType.bypass,
                    replica_groups=RG, ins=[kvown[:]], outs=[kvall[:]])
                for g in range(G):
                    nc.sync.dma_start(
                        kts[:, g], kvall[:, :, g].rearrange("c p w -> p c w"))
                nc.sync.dma_start(
                    vgs, kvall[:, :, 4:8].rearrange("c p i w -> p c i w"))

                # remaining Q heads overlap the collective
                for m in range(2, H):
                    q_head(m)

            # ---- Phase 2: attention, A/B software pipeline over heads.
            with tc.tile_pool(name="wop", bufs=2) as wop, \
                 tc.tile_pool(name="eap", bufs=3) as eap, \
                 tc.tile_pool(name="lip", bufs=2) as lip, \
                 tc.tile_pool(name="lbp", bufs=2) as lbp:
                won0 = wop.tile([P, EB, 512], bf16, tag="won")
                nc.scalar.dma_start(won0, wo[:, :, 0:512])  # prefetch phase 3

                with tc.tile_pool(name="pscp", bufs=2, space="PSUM") as pscp, \
                     tc.tile_pool(name="psop", bufs=2, space="PSUM") as psop, \
                     tc.tile_pool(name="pslp", bufs=2, space="PSUM") as pslp:
                    eas = [None, None, None]

                    def a_chunk(h, j, ea):
                        g = h // 4
                        ps2 = pscp.tile([P, 2, SQ], f32, tag="ps2")
                        for u in range(2):
                            t = 2 * j + u
                            nc.tensor.matmul(
                                ps2[:, u], kts[:, g, t * P:(t + 1) * P],
                                qts[:, h], start=True, stop=True)
                        nc.scalar.activation(ea[:, 2 * j:2 * j + 2], ps2, EXP)

                    def a_pass(h):
                        ea = eap.tile([P, EB, SQ], bf16, tag="ea")
                        for j in range(8):
                            a_chunk(h, j, ea)
                        eas[h % 3] = ea

                    def b_norm(h, pso, psl):
                        li = lip.tile([1, SQ], f32, tag="li")
                        nc.vector.reciprocal_approx_fast(li, psl)
                        lb = lbp.tile([P, SQ], f32, tag="lb")
                        nc.gpsimd.partition_broadcast(lb, li)
                        nc.vector.tensor_mul(OT[:, h], pso, lb)

                    # chunk-interleaved software pipeline: between every pair
                    # of (exp-gated) scores chunks sit four ready B matmuls,
                    # so the tensor engine always has dispatchable work.
                    a_pass(0)
                    for h in range(H):
                        g = h // 4
                        ea = eas[h % 3]
                        if h + 1 < H:
                            eanx = eap.tile([P, EB, SQ], bf16, tag="ea")
                        else:
                            eanx = None
                        pso = psop.tile([P, SQ], f32, tag="pso")
                        psl = pslp.tile([1, SQ], f32, tag="psl")
                        for j in range(8):
                            if eanx is not None:
                                a_chunk(h + 1, j, eanx)
                            for u in range(2):
                                t = 2 * j + u
                                nc.tensor.matmul(
                                    pso, vgs[:, t, g * P:(g + 1) * P], ea[:, t],
                                    start=(t == 0), stop=(t == EB - 1))
                                nc.tensor.matmul(
                                    psl, onec, ea[:, t],
                                    start=(t == 0), stop=(t == EB - 1))
                        if eanx is not None:
                            eas[(h + 1) % 3] = eanx
                        b_norm(h, pso, psl)

                # ---- Phase 3: output projection, contraction over the 16
                # head blocks; bias seeded via a K=1 ones matmul.
                with tc.tile_pool(name="obp", bufs=3) as obp, \
                     tc.tile_pool(name="ps3", bufs=2, space="PSUM") as ps3p:
                    for n in range(4):
                        if n == 0:
                            won = won0
                        else:
                            won = wop.tile([P, EB, 512], bf16, tag="won")
                            nc.scalar.dma_start(won, wo[:, :, 512 * n:512 * (n + 1)])
                        for ms in range(4):
                            ps = ps3p.tile([P, 512], f32, tag="ps")
                            nc.tensor.matmul(
                                ps, oner, bo_s[:, 512 * n:512 * (n + 1)],
                                start=True, stop=False)
                            for k in range(EB):
                                nc.tensor.matmul(
                                    ps, OT[:, k, ms * P:(ms + 1) * P],
                                    won[:, k],
                                    start=False, stop=(k == EB - 1))
                            ob = obp.tile([P, 512], f32, tag="ob")
                            nc.vector.tensor_copy(ob, ps)
                            nc.sync.dma_start(
                                out[ms * P:(ms + 1) * P, 512 * n:512 * (n + 1)], ob)

    nc.compile()
    return nc


def _get_nc():
    global _NC
    if _NC is None:
        _NC = _build()
    return _NC


def kernel(x, Wq, bq, Wkv, bkv, Wo, bo):
    from concourse.bass_utils import run_bass_kernel_spmd
    import ml_dtypes
    global LAST_RESULT

    bf = ml_dtypes.bfloat16
    x = np.asarray(x, np.float32)
    Wq = np.asarray(Wq, np.float32)
    bq = np.asarray(bq, np.float32)
    Wkv = np.asarray(Wkv, np.float32)
    bkv = np.asarray(bkv, np.float32)
    Wo = np.asarray(Wo, np.float32)
    bo = np.asarray(bo, np.float32)

    nc = _get_nc()
    sc = 1.0 / np.sqrt(E // H)
    wq_h = np.ascontiguousarray(
        (Wq * sc).reshape(EB, P, H, P).transpose(2, 1, 0, 3)).astype(bf)
    kcols = np.concatenate([Wkv[:, 256 * g:256 * g + 128] for g in range(G)], axis=1)
    vcols = np.concatenate([Wkv[:, 256 * g + 128:256 * g + 256] for g in range(G)], axis=1)
    wkv_re = np.concatenate([kcols, vcols], axis=1)  # [E, 1024]
    wkv_h = np.ascontiguousarray(wkv_re.reshape(EB, P, KV_N).transpose(1, 0, 2)).astype(bf)
    wo_h = np.ascontiguousarray(Wo.reshape(EB, P, E).transpose(1, 0, 2)).astype(bf)
    bq_h = np.ascontiguousarray((bq * sc).reshape(H, P).T).astype(np.float32)
    bkv_k = np.stack([bkv[256 * g:256 * g + 128] for g in range(G)], axis=1)
    bkv_v = np.concatenate([bkv[256 * g + 128:256 * g + 256] for g in range(G)])
    bkvk_h = np.ascontiguousarray(bkv_k).astype(np.float32)
    bkvv_h = np.ascontiguousarray(bkv_v.reshape(1, 512)).astype(bf)
    bo_h = np.ascontiguousarray(bo.reshape(1, E)).astype(bf)

    in_maps = []
    for c in range(NCORES):
        b, q = divmod(c, 4)
        xq = x[b, 512 * q:512 * (q + 1), :].T  # [e, s_own] — own quarter only
        xt_h = np.ascontiguousarray(
            xq.reshape(EB, P, SQ).transpose(1, 0, 2)).astype(bf)
        in_maps.append({"xt": xt_h, "wq": wq_h, "wkv": wkv_h, "wo": wo_h,
                        "bq": bq_h, "bkvk": bkvk_h, "bkvv": bkvv_h, "bo": bo_h})

    res = run_bass_kernel_spmd(nc, in_maps, core_ids=list(range(NCORES)),
                               trace=TRACE)
    LAST_RESULT = res

    outf = np.empty((2, S, E), np.float32)
    for c in range(NCORES):
        b, q = divmod(c, 4)
        outf[b, 512 * q:512 * (q + 1), :] = res.results[c]["out"]
    return outf



# revision 6
# speedup vs baseline: 1.2019x; 1.2019x over previous
"""Grouped Query Attention on 8 TRN2 NeuronCores (v3).

Sharding: batch x s_q-quarter (core c -> batch c//4, query rows
[512*(c%4), 512*(c%4+1))). Each core computes the Q projection for its
512 query rows, attention for all 16 heads over its query rows, and the
output projection for a disjoint [512, 2048] slice of the output.

KV: each core projects K^T and V for its OWN sequence quarter, packs
them into DRAM, and two 4-core AllGathers (K first, then V) assemble
the full K^T/V while the tensor engine runs the Q projection.

v3 structure:
- Phase order: K proj -> collective K -> V proj -> collective V ->
  Q heads 0..15. Collectives and gather DMAs fully overlap Q proj.
- wq preloaded per-head into a bufs=16 pool (no WAR stalls); gathers
  ride the scalar/gpsimd queues so the sync queue only carries the
  latency-critical x + wq stream.
- Softmax denominator: DVE tree-add over the 16 k-tiles + GPSIMD
  partition_all_reduce (no PE ones-matmuls, no partition_broadcast).
- Phase 2 processes head PAIRS (sharing the group's K/V stationary
  tiles); A-pass (scores+exp) of pair p+1 interleaves with B-pass
  (attnV) of pair p to keep the PE dense.
- Bias adds via DVE tensor_add with pre-broadcast bias rows (no K=1
  bias-seed matmuls).
- 1/sqrt(128) folded into Wq on host.
"""

import numpy as np

E = 2048
S = 2048
P = 128
H = 16
G = 4
SQ = 512          # query rows per core
EB = E // P       # 16 e-blocks (contraction tiles)
NCORES = 8

_NC = None
TRACE = False
LAST_RESULT = None


def _build():
    import concourse.bacc as bacc
    import concourse.mybir as mybir
    import concourse.tile as tile
    from concourse import bass_isa

    f32 = mybir.dt.float32
    bf16 = mybir.dt.bfloat16
    EXP = mybir.ActivationFunctionType.Exp
    IDENT = mybir.ActivationFunctionType.Identity

    nc = bacc.Bacc("TRN2", target_bir_lowering=False, debug=False,
                   num_devices=NCORES)

    # host layouts:
    #   xt:  x^T own quarter, [hd, eb, s_own]
    #   wq:  [head, p, eb, p] (1/sqrt(d) folded)
    #   wkv: [p, eb, 1024] with columns [K0 K1 K2 K3 V0 V1 V2 V3]
    #   wo:  [p, eb, e]
    xt = nc.declare_dram_parameter("xt", [P, EB, SQ], bf16, isOutput=False).ap()
    wq = nc.declare_dram_parameter("wq", [H, P, EB, P], bf16, isOutput=False).ap()
    wkv = nc.declare_dram_parameter("wkv", [P, EB, 2 * E // G], bf16, isOutput=False).ap()
    wo = nc.declare_dram_parameter("wo", [P, EB, E], bf16, isOutput=False).ap()
    bq = nc.declare_dram_parameter("bq", [P, H], f32, isOutput=False).ap()
    bkvk = nc.declare_dram_parameter("bkvk", [P, 4], f32, isOutput=False).ap()
    bkvv = nc.declare_dram_parameter("bkvv", [1, 512], f32, isOutput=False).ap()
    bo = nc.declare_dram_parameter("bo", [1, E], f32, isOutput=False).ap()
    out = nc.declare_dram_parameter("out", [SQ, E], f32, isOutput=True).ap()

    RG = [[0, 1, 2, 3], [4, 5, 6, 7]]

    with tile.TileContext(nc) as tc:
        with tc.tile_pool(name="consts", bufs=1) as cp, \
             tc.tile_pool(name="qtsp", bufs=1) as qtsp, \
             tc.tile_pool(name="kvp", bufs=1) as kvp, \
             tc.tile_pool(name="otp", bufs=1) as otp, \
             tc.tile_pool(name="dram", bufs=1, space="DRAM") as dp:
            # consts ride the scalar queue
            bq_s = cp.tile([P, H], f32, tag="bqs")
            nc.scalar.dma_start(bq_s, bq)
            bkvk_s = cp.tile([P, 4], f32, tag="bkvks")
            nc.scalar.dma_start(bkvk_s, bkvk)
            bkvv_b = cp.tile([P, 512], f32, tag="bkvvb")
            bo_b = cp.tile([P, E], f32, tag="bob")

            qts = qtsp.tile([P, H, SQ], bf16, tag="qts")    # Q^T, [hd, head, sq]
            kts = kvp.tile([P, G, S], bf16, tag="kts")      # K^T, [hd, group, sk]
            vgs = kvp.tile([P, EB, 512], bf16, tag="vgs")   # V, [sk, sk_tile, g*128+hd]
            OT = otp.tile([P, H, SQ], bf16, tag="ot")       # attn out, [hd, head, sq]

            # own-quarter KV packs + allgather landing buffers
            kvkown = dp.tile([P, 4, 512], bf16, tag="kvkown")
            kvkall = dp.tile([4, P, 4, 512], bf16, tag="kvkall")
            kvvown = dp.tile([P, 4, 512], bf16, tag="kvvown")
            kvvall = dp.tile([4, P, 4, 512], bf16, tag="kvvall")

            # ---- Phase 1: projections from the SBUF-resident x^T quarter.
            with tc.tile_pool(name="xsp", bufs=1) as xsp, \
                 tc.tile_pool(name="wkvp", bufs=1) as wkvp, \
                 tc.tile_pool(name="kvsg", bufs=1) as kvsg, \
                 tc.tile_pool(name="wqp", bufs=16) as wqp, \
                 tc.tile_pool(name="ps1", bufs=3, space="PSUM") as ps1, \
                 tc.tile_pool(name="ps1b", bufs=3, space="PSUM") as ps1b:
                # bias staging rows live only in phase-1 scope; the
                # broadcast copies (in cp) persist for phases 2/3.
                bkvv_s = kvsg.tile([1, 512], f32, tag="bkvvs")
                nc.scalar.dma_start(bkvv_s, bkvv)
                nc.gpsimd.partition_broadcast(bkvv_b, bkvv_s)
                bo_s = kvsg.tile([1, E], f32, tag="bos")
                nc.scalar.dma_start(bo_s, bo)
                nc.gpsimd.partition_broadcast(bo_b, bo_s)

                xs = xsp.tile([P, EB, SQ], bf16, tag="xs")
                # split so the first e-blocks land quickly (sync queue)
                for c4 in range(4):
                    nc.sync.dma_start(xs[:, 4 * c4:4 * (c4 + 1)],
                                      xt[:, 4 * c4:4 * (c4 + 1)])
                # wkv chunked by e-block so K proj can chase the DMA
                wkv_s = wkvp.tile([P, EB, 2 * E // G], bf16, tag="wkvs")
                for c4 in range(4):
                    nc.scalar.dma_start(wkv_s[:, 4 * c4:4 * (c4 + 1)],
                                        wkv[:, 4 * c4:4 * (c4 + 1)])
                # all 16 Q-head weights preloaded (sync queue, fresh bufs)
                wq_s = []
                for m in range(H):
                    wqm = wqp.tile([P, EB, P], bf16, tag="wqm")
                    nc.sync.dma_start(wqm, wq[m])
                    wq_s.append(wqm)

                kvstg_k = kvsg.tile([P, 4, 512], bf16, tag="kvstgk")
                kvstg_v = kvsg.tile([P, 4, 512], bf16, tag="kvstgv")

                # K^T for all 4 groups over this core's own quarter
                for m in range(G):
                    ps = ps1b.tile([P, 512], f32, tag="ps")
                    for b in range(EB):
                        nc.tensor.matmul(
                            ps, wkv_s[:, b, m * P:(m + 1) * P], xs[:, b],
                            start=(b == 0), stop=(b == EB - 1))
                    nc.scalar.activation(kvstg_k[:, m], ps, IDENT,
                                         bias=bkvk_s[:, m:m + 1])
                nc.gpsimd.dma_start(kvkown, kvstg_k)
                nc.gpsimd.collective_compute(
                    "AllGather", mybir.AluOpType.bypass,
                    replica_groups=RG, ins=[kvkown[:]], outs=[kvkall[:]])
                # K gathers on the scalar queue (idle; wq stream unaffected)
                for g in range(G):
                    nc.scalar.dma_start(
                        kts[:, g], kvkall[:, :, g].rearrange("c p w -> p c w"))

                # V in [s, d] orientation for this core's own 4 s-tiles
                for t in range(4):
                    ps = ps1b.tile([P, 512], f32, tag="ps")
                    for b in range(EB):
                        nc.tensor.matmul(
                            ps, xs[:, b, t * P:(t + 1) * P],
                            wkv_s[:, b, 512:2 * E // G],
                            start=(b == 0), stop=(b == EB - 1))
                    nc.vector.tensor_add(kvstg_v[:, t], ps, bkvv_b)
                nc.gpsimd.dma_start(kvvown, kvstg_v)
                nc.gpsimd.collective_compute(
                    "AllGather", mybir.AluOpType.bypass,
                    replica_groups=RG, ins=[kvvown[:]], outs=[kvvall[:]])
                nc.gpsimd.dma_start(
                    vgs, kvvall.rearrange("c p i w -> p c i w"))

                # Q heads; collectives + gathers overlap this
                for m in range(H):
                    ps = ps1.tile([P, SQ], f32, tag="ps")
                    for b in range(EB):
                        nc.tensor.matmul(ps, wq_s[m][:, b], xs[:, b],
                                         start=(b == 0), stop=(b == EB - 1))
                    nc.vector.tensor_scalar_add(qts[:, m], ps, bq_s[:, m:m + 1])

            # ---- Phase 2: attention over head pairs, A/B software pipeline.
            with tc.tile_pool(name="wop", bufs=2) as wop, \
                 tc.tile_pool(name="eap", bufs=2) as eap, \
                 tc.tile_pool(name="r1p", bufs=1) as r1p, \
                 tc.tile_pool(name="rsp", bufs=1) as rsp, \
                 tc.tile_pool(name="dbp", bufs=1) as dbp, \
                 tc.tile_pool(name="rbp", bufs=1) as rbp:
                won0 = wop.tile([P, EB, 512], bf16, tag="won")
                nc.sync.dma_start(won0, wo[:, :, 0:512])  # prefetch phase 3

                with tc.tile_pool(name="pscp", bufs=3, space="PSUM") as pscp, \
                     tc.tile_pool(name="psop", bufs=2, space="PSUM") as psop:

                    def a_step(h0, g, t, ea):
                        ps2 = pscp.tile([P, 2, SQ], f32, tag="ps2")
                        nc.tensor.matmul(ps2[:, 0],
                                         kts[:, g, t * P:(t + 1) * P],
                                         qts[:, h0], start=True, stop=True)
                        nc.tensor.matmul(ps2[:, 1],
                                         kts[:, g, t * P:(t + 1) * P],
                                         qts[:, h0 + 1], start=True, stop=True)
                        nc.scalar.activation(ea[:, t], ps2, EXP)

                    # prologue: full A-pass for pair 0
                    ea_cur = eap.tile([P, EB, 2, SQ], bf16, tag="ea")
                    for t in range(EB):
                        a_step(0, 0, t, ea_cur)

                    for p in range(8):
                        g = p // 2
                        h0 = 2 * p
                        if p < 7:
                            ea_nxt = eap.tile([P, EB, 2, SQ], bf16, tag="ea")
                        else:
                            ea_nxt = None
                        # denominator: DVE tree over the 16 k-tiles, then
                        # gpsimd all-reduce over partitions (result is
                        # already broadcast), then DVE reciprocal.
                        r1 = r1p.tile([P, 8, 2, SQ], bf16, tag="r1")
                        nc.vector.tensor_add(r1, ea_cur[:, 0:8], ea_cur[:, 8:16])
                        nc.vector.tensor_add(r1[:, 0:4], r1[:, 0:4], r1[:, 4:8])
                        nc.vector.tensor_add(r1[:, 0:2], r1[:, 0:2], r1[:, 2:4])
                        rs = rsp.tile([P, 2, SQ], bf16, tag="rs")
                        nc.vector.tensor_add(rs, r1[:, 0], r1[:, 1])
                        den = dbp.tile([P, 2, SQ], f32, tag="den")
                        nc.gpsimd.partition_all_reduce(
                            den, rs, channels=P,
                            reduce_op=bass_isa.ReduceOp.add)
                        rec = rbp.tile([P, 2, SQ], f32, tag="rec")
                        nc.vector.reciprocal_approx_fast(rec, den)

                        pso0 = psop.tile([P, SQ], f32, tag="pso")
                        pso1 = psop.tile([P, SQ], f32, tag="pso")
                        for t in range(EB):
                            if ea_nxt is not None:
                                a_step(2 * (p + 1), (p + 1) // 2, t, ea_nxt)
                            nc.tensor.matmul(
                                pso0, vgs[:, t, g * P:(g + 1) * P],
                                ea_cur[:, t, 0],
                                start=(t == 0), stop=(t == EB - 1))
                            nc.tensor.matmul(
                                pso1, vgs[:, t, g * P:(g + 1) * P],
                                ea_cur[:, t, 1],
                                start=(t == 0), stop=(t == EB - 1))
                        nc.vector.tensor_mul(OT[:, h0], pso0, rec[:, 0])
                        nc.vector.tensor_mul(OT[:, h0 + 1], pso1, rec[:, 1])
                        ea_cur = ea_nxt

                # ---- Phase 3: output projection, contraction over the 16
                # head blocks; bias added on the PSUM->SBUF copy.
                with tc.tile_pool(name="obp", bufs=2) as obp, \
                     tc.tile_pool(name="ps3", bufs=2, space="PSUM") as ps3p:
                    wons = [won0]
                    for n in range(4):
                        if n + 1 < 4:
                            wnx = wop.tile([P, EB, 512], bf16, tag="won")
                            nc.sync.dma_start(
                                wnx, wo[:, :, 512 * (n + 1):512 * (n + 2)])
                            wons.append(wnx)
                        won = wons[n]
                        for ms in range(4):
                            ps = ps3p.tile([P, 512], f32, tag="ps")
                            for k in range(EB):
                                nc.tensor.matmul(
                                    ps, OT[:, k, ms * P:(ms + 1) * P],
                                    won[:, k],
                                    start=(k == 0), stop=(k == EB - 1))
                            ob = obp.tile([P, 512], f32, tag="ob")
                            nc.vector.tensor_add(
                                ob, ps, bo_b[:, 512 * n:512 * (n + 1)])
                            nc.sync.dma_start(
                                out[ms * P:(ms + 1) * P, 512 * n:512 * (n + 1)], ob)

    nc.compile()
    return nc


def _get_nc():
    global _NC
    if _NC is None:
        _NC = _build()
    return _NC


def kernel(x, Wq, bq, Wkv, bkv, Wo, bo):
    from concourse.bass_utils import run_bass_kernel_spmd
    import ml_dtypes
    global LAST_RESULT

    bf = ml_dtypes.bfloat16
    x = np.asarray(x, np.float32)
    Wq = np.asarray(Wq, np.float32)
    bq = np.asarray(bq, np.float32)
    Wkv = np.asarray(Wkv, np.float32)
    bkv = np.asarray(bkv, np.float32)
    Wo = np.asarray(Wo, np.float32)
    bo = np.asarray(bo, np.float32)

    nc = _get_nc()
    sc = 1.0 / np.sqrt(E // H)
    wq_h = np.ascontiguousarray(
        (Wq * sc).reshape(EB, P, H, P).transpose(2, 1, 0, 3)).astype(bf)
    kcols = np.concatenate([Wkv[:, 256 * g:256 * g + 128] for g in range(G)], axis=1)
    vcols = np.concatenate([Wkv[:, 256 * g + 128:256 * g + 256] for g in range(G)], axis=1)
    wkv_re = np.concatenate([kcols, vcols], axis=1)  # [E, 1024]
    wkv_h = np.ascontiguousarray(
        wkv_re.reshape(EB, P, 2 * E // G).transpose(1, 0, 2)).astype(bf)
    wo_h = np.ascontiguousarray(Wo.reshape(EB, P, E).transpose(1, 0, 2)).astype(bf)
    bq_h = np.ascontiguousarray((bq * sc).reshape(H, P).T).astype(np.float32)
    bkv_k = np.stack([bkv[256 * g:256 * g + 128] for g in range(G)], axis=1)
    bkv_v = np.concatenate([bkv[256 * g + 128:256 * g + 256] for g in range(G)])
    bkvk_h = np.ascontiguousarray(bkv_k).astype(np.float32)
    bkvv_h = np.ascontiguousarray(bkv_v.reshape(1, 512)).astype(np.float32)
    bo_h = np.ascontiguousarray(bo.reshape(1, E)).astype(np.float32)

    in_maps = []
    for c in range(NCORES):
        b, q = divmod(c, 4)
        xq = x[b, 512 * q:512 * (q + 1), :].T  # [e, s_own] — own quarter only
        xt_h = np.ascontiguousarray(
            xq.reshape(EB, P, SQ).transpose(1, 0, 2)).astype(bf)
        in_maps.append({"xt": xt_h, "wq": wq_h, "wkv": wkv_h, "wo": wo_h,
                        "bq": bq_h, "bkvk": bkvk_h, "bkvv": bkvv_h, "bo": bo_h})

    res = run_bass_kernel_spmd(nc, in_maps, core_ids=list(range(NCORES)),
                               trace=TRACE)
    LAST_RESULT = res

    outf = np.empty((2, S, E), np.float32)
    for c in range(NCORES):
        b, q = divmod(c, 4)
        outf[b, 512 * q:512 * (q + 1), :] = res.results[c]["out"]
    return outf


# revision 13
# speedup vs baseline: 1.2599x; 1.0483x over previous
"""Grouped Query Attention on 8 TRN2 NeuronCores (v3).

Sharding: batch x s_q-quarter (core c -> batch c//4, query rows
[512*(c%4), 512*(c%4+1))). Each core computes the Q projection for its
512 query rows, attention for all 16 heads over its query rows, and the
output projection for a disjoint [512, 2048] slice of the output.

KV: each core projects K^T and V for its OWN sequence quarter, packs
them into DRAM, and two 4-core AllGathers (K first, then V) assemble
the full K^T/V while the tensor engine runs the Q projection.

v3 structure:
- Phase order: K proj -> collective K -> V proj -> collective V ->
  Q heads 0..15. Collectives and gather DMAs fully overlap Q proj.
- wq preloaded per-head into a bufs=16 pool (no WAR stalls); gathers
  ride the scalar/gpsimd queues so the sync queue only carries the
  latency-critical x + wq stream.
- Softmax denominator: DVE tree-add over the 16 k-tiles + GPSIMD
  partition_all_reduce (no PE ones-matmuls, no partition_broadcast).
- Phase 2 processes head PAIRS (sharing the group's K/V stationary
  tiles); A-pass (scores+exp) of pair p+1 interleaves with B-pass
  (attnV) of pair p to keep the PE dense.
- Bias adds via DVE tensor_add with pre-broadcast bias rows (no K=1
  bias-seed matmuls).
- 1/sqrt(128) folded into Wq on host.
"""

import numpy as np

E = 2048
S = 2048
P = 128
H = 16
G = 4
SQ = 512          # query rows per core
EB = E // P       # 16 e-blocks (contraction tiles)
NCORES = 8

_NC = None
TRACE = False
LAST_RESULT = None


def _build():
    import concourse.bacc as bacc
    import concourse.mybir as mybir
    import concourse.tile as tile
    from concourse import bass_isa

    f32 = mybir.dt.float32
    bf16 = mybir.dt.bfloat16
    EXP = mybir.ActivationFunctionType.Exp
    IDENT = mybir.ActivationFunctionType.Identity

    nc = bacc.Bacc("TRN2", target_bir_lowering=False, debug=False,
                   num_devices=NCORES)

    # host layouts:
    #   xt:  x^T own quarter, [hd, eb, s_own]
    #   wq:  [head, p, eb, p] (1/sqrt(d) folded)
    #   wkv: [p, eb, 1024] with columns [K0 K1 K2 K3 V0 V1 V2 V3]
    #   wo:  [p, eb, e]
    xt = nc.declare_dram_parameter("xt", [P, EB, SQ], bf16, isOutput=False).ap()
    wq = nc.declare_dram_parameter("wq", [H, P, EB, P], bf16, isOutput=False).ap()
    wkv = nc.declare_dram_parameter("wkv", [P, EB, 2 * E // G], bf16, isOutput=False).ap()
    wo = nc.declare_dram_parameter("wo", [P, EB, E], bf16, isOutput=False).ap()
    bq = nc.declare_dram_parameter("bq", [P, H], f32, isOutput=False).ap()
    bkvk = nc.declare_dram_parameter("bkvk", [P, 4], f32, isOutput=False).ap()
    bkvv = nc.declare_dram_parameter("bkvv", [1, 512], f32, isOutput=False).ap()
    bo = nc.declare_dram_parameter("bo", [1, E], f32, isOutput=False).ap()
    out = nc.declare_dram_parameter("out", [SQ, E], f32, isOutput=True).ap()

    RG = [[0, 1, 2, 3], [4, 5, 6, 7]]

    with tile.TileContext(nc) as tc:
        with tc.tile_pool(name="consts", bufs=1) as cp, \
             tc.tile_pool(name="qtsp", bufs=1) as qtsp, \
             tc.tile_pool(name="kvp", bufs=1) as kvp, \
             tc.tile_pool(name="otp", bufs=1) as otp, \
             tc.tile_pool(name="dram", bufs=1, space="DRAM") as dp:
            # consts ride the scalar queue
            bq_s = cp.tile([P, H], f32, tag="bqs")
            nc.scalar.dma_start(bq_s, bq)
            bkvk_s = cp.tile([P, 4], f32, tag="bkvks")
            nc.scalar.dma_start(bkvk_s, bkvk)
            bkvv_b = cp.tile([P, 512], f32, tag="bkvvb")
            bo_b = cp.tile([P, E], f32, tag="bob")

            qts = qtsp.tile([P, H, SQ], bf16, tag="qts")    # Q^T, [hd, head, sq]
            kts = kvp.tile([P, G, S], bf16, tag="kts")      # K^T, [hd, group, sk]
            vgs = kvp.tile([P, EB, 512], bf16, tag="vgs")   # V, [sk, sk_tile, g*128+hd]
            OT = otp.tile([P, H, SQ], bf16, tag="ot")       # attn out, [hd, head, sq]

            # own-quarter KV packs + allgather landing buffers
            kvkown = dp.tile([P, 4, 512], bf16, tag="kvkown")
            kvkall = dp.tile([4, P, 4, 512], bf16, tag="kvkall")
            kvvown = dp.tile([P, 4, 512], bf16, tag="kvvown")
            kvvall = dp.tile([4, P, 4, 512], bf16, tag="kvvall")

            # ---- Phase 1: projections from the SBUF-resident x^T quarter.
            with tc.tile_pool(name="xsp", bufs=1) as xsp, \
                 tc.tile_pool(name="wkvp", bufs=1) as wkvp, \
                 tc.tile_pool(name="kvsg", bufs=1) as kvsg, \
                 tc.tile_pool(name="wqp", bufs=16) as wqp, \
                 tc.tile_pool(name="ps1", bufs=3, space="PSUM") as ps1, \
                 tc.tile_pool(name="ps1b", bufs=3, space="PSUM") as ps1b:
                # wkv first on the scalar queue: K proj is the first PE work
                wkv_s = wkvp.tile([P, EB, 2 * E // G], bf16, tag="wkvs")
                for c4 in range(4):
                    nc.scalar.dma_start(wkv_s[:, 4 * c4:4 * (c4 + 1)],
                                        wkv[:, 4 * c4:4 * (c4 + 1)])
                # bias staging rows live only in phase-1 scope; the
                # broadcast copies (in cp) persist for phases 2/3.
                bkvv_s = kvsg.tile([1, 512], f32, tag="bkvvs")
                nc.scalar.dma_start(bkvv_s, bkvv)
                nc.gpsimd.partition_broadcast(bkvv_b, bkvv_s)
                bo_s = kvsg.tile([1, E], f32, tag="bos")
                nc.scalar.dma_start(bo_s, bo)
                nc.gpsimd.partition_broadcast(bo_b, bo_s)

                # x on sync; wq split across sync and gpsimd queues (a
                # single DMA queue sustains only ~90-130 GB/s)
                xs = xsp.tile([P, EB, SQ], bf16, tag="xs")
                for c4 in range(4):
                    nc.sync.dma_start(xs[:, 4 * c4:4 * (c4 + 1)],
                                      xt[:, 4 * c4:4 * (c4 + 1)])
                wq_s = []
                for m in range(H):
                    wqm = wqp.tile([P, EB, P], bf16, tag="wqm")
                    if m % 2 == 0:
                        nc.sync.dma_start(wqm, wq[m])
                    else:
                        nc.gpsimd.dma_start(wqm, wq[m])
                    wq_s.append(wqm)

                kvstg_k = kvsg.tile([P, 4, 512], bf16, tag="kvstgk")
                kvstg_v = kvsg.tile([P, 4, 512], bf16, tag="kvstgv")

                # K^T for all 4 groups over this core's own quarter
                for m in range(G):
                    ps = ps1b.tile([P, 512], f32, tag="ps")
                    for b in range(EB):
                        nc.tensor.matmul(
                            ps, wkv_s[:, b, m * P:(m + 1) * P], xs[:, b],
                            start=(b == 0), stop=(b == EB - 1))
                    nc.scalar.activation(kvstg_k[:, m], ps, IDENT,
                                         bias=bkvk_s[:, m:m + 1])
                nc.gpsimd.dma_start(kvkown, kvstg_k)
                nc.gpsimd.collective_compute(
                    "AllGather", mybir.AluOpType.bypass,
                    replica_groups=RG, ins=[kvkown[:]], outs=[kvkall[:]])
                # per-source-core contiguous K gathers on the scalar queue
                for c in range(4):
                    nc.scalar.dma_start(
                        kts[:, :, 512 * c:512 * (c + 1)], kvkall[c])

                # V in [s, d] orientation for this core's own 4 s-tiles
                for t in range(4):
                    ps = ps1b.tile([P, 512], f32, tag="ps")
                    for b in range(EB):
                        nc.tensor.matmul(
                            ps, xs[:, b, t * P:(t + 1) * P],
                            wkv_s[:, b, 512:2 * E // G],
                            start=(b == 0), stop=(b == EB - 1))
                    nc.vector.tensor_add(kvstg_v[:, t], ps, bkvv_b)
                nc.gpsimd.dma_start(kvvown, kvstg_v)
                nc.gpsimd.collective_compute(
                    "AllGather", mybir.AluOpType.bypass,
                    replica_groups=RG, ins=[kvvown[:]], outs=[kvvall[:]])
                # per-source-core contiguous V gathers, two queues
                for c in range(4):
                    eng = nc.gpsimd if c % 2 == 0 else nc.scalar
                    eng.dma_start(vgs[:, 4 * c:4 * (c + 1)], kvvall[c])

                # Q heads; collectives + gathers overlap this
                for m in range(H):
                    ps = ps1.tile([P, SQ], f32, tag="ps")
                    for b in range(EB):
                        nc.tensor.matmul(ps, wq_s[m][:, b], xs[:, b],
                                         start=(b == 0), stop=(b == EB - 1))
                    nc.vector.tensor_scalar_add(qts[:, m], ps, bq_s[:, m:m + 1])

            # ---- Phase 2: attention over head pairs, A/B software pipeline.
            with tc.tile_pool(name="wop", bufs=2) as wop, \
                 tc.tile_pool(name="eap", bufs=2) as eap, \
                 tc.tile_pool(name="r1p", bufs=1) as r1p, \
                 tc.tile_pool(name="rsp", bufs=1) as rsp, \
                 tc.tile_pool(name="dbp", bufs=2) as dbp, \
                 tc.tile_pool(name="rbp", bufs=1) as rbp:
                won0 = wop.tile([P, EB, 512], bf16, tag="won")
                nc.sync.dma_start(won0, wo[:, :, 0:512])  # prefetch phase 3

                with tc.tile_pool(name="pscp", bufs=2, space="PSUM") as pscp, \
                     tc.tile_pool(name="psop", bufs=4, space="PSUM") as psop:

                    def a_step(h0, g, t, ea):
                        ps2 = pscp.tile([P, 2, SQ], f32, tag="ps2")
                        nc.tensor.matmul(ps2[:, 0],
                                         kts[:, g, t * P:(t + 1) * P],
                                         qts[:, h0], start=True, stop=True)
                        nc.tensor.matmul(ps2[:, 1],
                                         kts[:, g, t * P:(t + 1) * P],
                                         qts[:, h0 + 1], start=True, stop=True)
                        nc.scalar.activation(ea[:, t], ps2, EXP)

                    # prologue: full A-pass for pair 0
                    ea_cur = eap.tile([P, EB, 2, SQ], bf16, tag="ea")
                    for t in range(EB):
                        a_step(0, 0, t, ea_cur)

                    # denominator chain for pair p: DVE tree over the 16
                    # k-tiles -> gpsimd partition all-reduce (result already
                    # broadcast) -> DVE reciprocal -> DVE normalize into OT.
                    # recip+mul of pair p are issued in iteration p+1 so the
                    # DVE queue never blocks on the in-flight all-reduce.
                    recs = [None] * 8
                    psos = [None] * 8

                    def denom_front(p, ea):
                        r1 = r1p.tile([P, 8, 2, SQ], bf16, tag="r1")
                        nc.vector.tensor_add(r1, ea[:, 0:8], ea[:, 8:16])
                        nc.vector.tensor_add(r1[:, 0:4], r1[:, 0:4], r1[:, 4:8])
                        nc.vector.tensor_add(r1[:, 0:2], r1[:, 0:2], r1[:, 2:4])
                        rs = rsp.tile([P, 2, SQ], bf16, tag="rs")
                        nc.vector.tensor_add(rs, r1[:, 0], r1[:, 1])
                        den = dbp.tile([P, 2, SQ], f32, tag="den")
                        nc.gpsimd.partition_all_reduce(
                            den, rs, channels=P,
                            reduce_op=bass_isa.ReduceOp.add)
                        rec = rbp.tile([P, 2, SQ], f32, tag="rec")
                        recs[p] = (den, rec)

                    def denom_back(p):
                        den, rec = recs[p]
                        nc.vector.reciprocal_approx_fast(rec, den)
                        pso0, pso1 = psos[p]
                        nc.vector.tensor_mul(OT[:, 2 * p], pso0, rec[:, 0])
                        nc.vector.tensor_mul(OT[:, 2 * p + 1], pso1, rec[:, 1])

                    for p in range(8):
                        g = p // 2
                        if p < 7:
                            ea_nxt = eap.tile([P, EB, 2, SQ], bf16, tag="ea")
                        else:
                            ea_nxt = None
                        denom_front(p, ea_cur)
                        if p > 0:
                            denom_back(p - 1)
                        pso0 = psop.tile([P, SQ], f32, tag="pso")
                        pso1 = psop.tile([P, SQ], f32, tag="pso")
                        psos[p] = (pso0, pso1)
                        for t in range(EB):
                            if ea_nxt is not None:
                                a_step(2 * (p + 1), (p + 1) // 2, t, ea_nxt)
                            nc.tensor.matmul(
                                pso0, vgs[:, t, g * P:(g + 1) * P],
                                ea_cur[:, t, 0],
                                start=(t == 0), stop=(t == EB - 1))
                            nc.tensor.matmul(
                                pso1, vgs[:, t, g * P:(g + 1) * P],
                                ea_cur[:, t, 1],
                                start=(t == 0), stop=(t == EB - 1))
                        ea_cur = ea_nxt
                    denom_back(7)

                # ---- Phase 3: output projection, contraction over the 16
                # head blocks; bias added on the PSUM->SBUF copy.
                with tc.tile_pool(name="obp", bufs=2) as obp, \
                     tc.tile_pool(name="ps3", bufs=2, space="PSUM") as ps3p:
                    wons = [won0]
                    for n in range(4):
                        if n + 1 < 4:
                            wnx = wop.tile([P, EB, 512], bf16, tag="won")
                            nc.sync.dma_start(
                                wnx, wo[:, :, 512 * (n + 1):512 * (n + 2)])
                            wons.append(wnx)
                        won = wons[n]
                        for ms in range(4):
                            ps = ps3p.tile([P, 512], f32, tag="ps")
                            for k in range(EB):
                                nc.tensor.matmul(
                                    ps, OT[:, k, ms * P:(ms + 1) * P],
                                    won[:, k],
                                    start=(k == 0), stop=(k == EB - 1))
                            ob = obp.tile([P, 512], f32, tag="ob")
                            nc.vector.tensor_add(
                                ob, ps, bo_b[:, 512 * n:512 * (n + 1)])
                            nc.sync.dma_start(
                                out[ms * P:(ms + 1) * P, 512 * n:512 * (n + 1)], ob)

    nc.compile()
    return nc


def _get_nc():
    global _NC
    if _NC is None:
        _NC = _build()
    return _NC


def kernel(x, Wq, bq, Wkv, bkv, Wo, bo):
    from concourse.bass_utils import run_bass_kernel_spmd
    import ml_dtypes
    global LAST_RESULT

    bf = ml_dtypes.bfloat16
    x = np.asarray(x, np.float32)
    Wq = np.asarray(Wq, np.float32)
    bq = np.asarray(bq, np.float32)
    Wkv = np.asarray(Wkv, np.float32)
    bkv = np.asarray(bkv, np.float32)
    Wo = np.asarray(Wo, np.float32)
    bo = np.asarray(bo, np.float32)

    nc = _get_nc()
    sc = 1.0 / np.sqrt(E // H)
    wq_h = np.ascontiguousarray(
        (Wq * sc).reshape(EB, P, H, P).transpose(2, 1, 0, 3)).astype(bf)
    kcols = np.concatenate([Wkv[:, 256 * g:256 * g + 128] for g in range(G)], axis=1)
    vcols = np.concatenate([Wkv[:, 256 * g + 128:256 * g + 256] for g in range(G)], axis=1)
    wkv_re = np.concatenate([kcols, vcols], axis=1)  # [E, 1024]
    wkv_h = np.ascontiguousarray(
        wkv_re.reshape(EB, P, 2 * E // G).transpose(1, 0, 2)).astype(bf)
    wo_h = np.ascontiguousarray(Wo.reshape(EB, P, E).transpose(1, 0, 2)).astype(bf)
    bq_h = np.ascontiguousarray((bq * sc).reshape(H, P).T).astype(np.float32)
    bkv_k = np.stack([bkv[256 * g:256 * g + 128] for g in range(G)], axis=1)
    bkv_v = np.concatenate([bkv[256 * g + 128:256 * g + 256] for g in range(G)])
    bkvk_h = np.ascontiguousarray(bkv_k).astype(np.float32)
    bkvv_h = np.ascontiguousarray(bkv_v.reshape(1, 512)).astype(np.float32)
    bo_h = np.ascontiguousarray(bo.reshape(1, E)).astype(np.float32)

    in_maps = []
    for c in range(NCORES):
        b, q = divmod(c, 4)
        xq = x[b, 512 * q:512 * (q + 1), :].T  # [e, s_own] — own quarter only
        xt_h = np.ascontiguousarray(
            xq.reshape(EB, P, SQ).transpose(1, 0, 2)).astype(bf)
        in_maps.append({"xt": xt_h, "wq": wq_h, "wkv": wkv_h, "wo": wo_h,
                        "bq": bq_h, "bkvk": bkvk_h, "bkvv": bkvv_h, "bo": bo_h})

    res = run_bass_kernel_spmd(nc, in_maps, core_ids=list(range(NCORES)),
                               trace=TRACE)
    LAST_RESULT = res

    outf = np.empty((2, S, E), np.float32)
    for c in range(NCORES):
        b, q = divmod(c, 4)
        outf[b, 512 * q:512 * (q + 1), :] = res.results[c]["out"]
    return outf
